# revision 1
# baseline (speedup 1.0000x reference)
"""CommonNeighborsPredictor kernel for 8 Trainium2 NeuronCores.

Math (see reference):
    deg = adj.sum(-1) + 1e-6
    x   = emb + (adj @ emb) / deg[:, None]
    xn  = x / max(||x||_2, 1e-8)                            # row-normalize
    w_e = sum_c adj[src_e, c] * adj[dst_e, c] * (xn[src_e]@xn[c]) * (xn[dst_e]@xn[c])
    out = sigmoid(w)

Distribution (2 SPMD launches, no collectives):
  Stage 1: shard nodes (rows of adj) 8 ways. Core k computes xn for its
    1250 nodes.  The matmul contracts over adj columns, so the host feeds
    adj[rows_k,:].T (k-major, bf16 - adjacency 0/1 values are exact) and
    the kernel computes xn TRANSPOSED ([256, 1250]) which is the layout
    stage 2 wants.  The k-loop is outermost: one wide DMA per k-tile feeds
    6 accumulating PSUM tiles (2 d-chunks x 3 m-chunks); degrees are
    accumulated on DVE (0/1 sums are exact in bf16) and reduced across
    partitions with an M=1 ones matmul.  Per-node scalars (1/deg, 1/norm)
    are broadcast across partitions with K=1 ones matmuls.  Host
    concatenates the shards -> xnT [256, 10000] (bf16).
  Stage 2: shard query edges 8 ways (512 each).  Core k gathers whole adj
    rows for its edges out of a per-core dedup'd row table via one
    indirect DMA per edge-tile per side; the src*dst mask product runs on
    GPSIMD (in place).  cos tiles accumulate into 2-bank PSUM pairs from
    PE matmuls against resident xnT; DVE does the two mask/cos products,
    the scalar engine row-sums them via activation accum_out, and applies
    the final sigmoid.  Host concatenates the 8 edge shards.

dtypes: all matmul operands and adjacency data are bf16 (adjacency is
exact; emb/xn rounding contributes ~3e-5 max output error vs the fp32
reference).  PSUM accumulation and the normalization epilogue are fp32.
"""

import numpy as np

import concourse.bass as bass
import concourse.bacc as bacc
import concourse.mybir as mybir
import concourse.tile as tile
from concourse import bass_utils

F32 = mybir.dt.float32
BF16 = mybir.dt.bfloat16
I32 = mybir.dt.int32
AF = mybir.ActivationFunctionType
OP = mybir.AluOpType
NP_BF16 = mybir.dt.np(BF16)

N, D, Q, NC = 10000, 256, 4096, 8

# bf16 for matmul operands and the 0/1 adjacency data (adjacency values are
# exact in bf16); accumulation/epilogue stay fp32.
USE_BF16 = True


def _chunks(total, step):
    return [(s, min(step, total - s)) for s in range(0, total, step)]


def build_stage1(n=N, d=D, nc_cores=NC, mm_dt=F32, out_dt=F32):
    """Per-core: xnT_shard [d, n/nc] from adjT shard + emb."""
    msh = n // nc_cores
    kt = (n + 127) // 128
    kp = kt * 128
    dst = d + 1  # emb columns + ones column (for degrees)
    ndt = d // 128

    b = bacc.Bacc("TRN2", target_bir_lowering=False, debug=False, num_devices=nc_cores)
    adjT = b.dram_tensor("adjT", [kp, msh], mm_dt, kind="ExternalInput")
    embx = b.dram_tensor("embx", [128, kt * dst], mm_dt, kind="ExternalInput")
    embT = b.dram_tensor("embT", [d, msh], F32, kind="ExternalInput")
    xnT = b.dram_tensor("xnT", [d, msh], out_dt, kind="ExternalOutput")

    mchunks = _chunks(msh, 512)
    with tile.TileContext(b) as tc:
        with (
            tc.tile_pool(name="const", bufs=1) as cpool,
            tc.tile_pool(name="stream", bufs=4) as spool,
            tc.tile_pool(name="work", bufs=2) as wpool,
            tc.tile_pool(name="acc", bufs=1, space="PSUM") as apool,
            tc.tile_pool(name="bc", bufs=1, space="PSUM") as bpool,
        ):
            EKT = 10  # k-tiles per emb chunk tile
            emb_chunks = _chunks(kt, EKT)
            emb_sb_l = [None] * len(emb_chunks)

            def load_emb_chunk(ci):
                t0, tw = emb_chunks[ci]
                e_ = cpool.tile([128, tw * dst], mm_dt, tag=f"emb{t0}", name=f"emb{t0}")
                b.sync.dma_start(
                    out=e_[:], in_=embx.ap()[:, t0 * dst : (t0 + tw) * dst]
                )
                emb_sb_l[ci] = e_

            def emb_sl(t, lo, hi):
                e_ = emb_sb_l[t // EKT]
                base = (t % EKT) * dst
                return e_[:, base + lo : base + hi]

            at_tiles = {}

            def at_dma(t):
                a_ = spool.tile(
                    [128, msh], mm_dt, tag="adjT", bufs=6, name=f"at{t}"
                )
                b.sync.dma_start(
                    out=a_[:], in_=adjT.ap()[128 * t : 128 * (t + 1), :]
                )
                at_tiles[t] = a_

            # issue order: first emb chunk, a few adjT tiles (so PE starts
            # ~immediately), then the rest of emb
            load_emb_chunk(0)
            for t in range(min(6, kt)):
                at_dma(t)
            for ci in range(1, len(emb_chunks)):
                load_emb_chunk(ci)
            ones_row = cpool.tile([1, 128], F32)
            b.vector.memset(ones_row[:1, :], 1.0)
            ones_col = cpool.tile([128, 1], F32)
            b.vector.memset(ones_col[:, :1], 1.0)
            ones_col_mm = cpool.tile([128, 1], mm_dt)
            b.vector.memset(ones_col_mm[:, :1], 1.0)

            # k-outer loop: one wide DMA per k-tile; 2 n-chunks x m-chunks of
            # PSUM accumulate; degrees accumulated on DVE (0/1 sums are exact
            # in bf16 too).
            ps_y = {
                (i, m0): apool.tile([128, mw], F32, tag=f"py{i}_{m0}", name=f"py{i}_{m0}")
                for i in range(ndt)
                for (m0, mw) in mchunks
            }
            NDEG = 4  # independent partial chains so the adds pipeline
            deg_p = [
                cpool.tile([128, msh], mm_dt, tag=f"degp{j}", name=f"degp{j}")
                for j in range(NDEG)
            ]
            for t in range(kt):
                if t not in at_tiles:
                    at_dma(t)
                at = at_tiles.pop(t)
                j = t % NDEG
                if t < NDEG:
                    b.vector.tensor_copy(deg_p[j][:], at[:])
                else:
                    b.vector.tensor_add(deg_p[j][:], deg_p[j][:], at[:])
                st, sp = (t == 0), (t == kt - 1)
                for i in range(ndt):
                    for (m0, mw) in mchunks:
                        b.tensor.matmul(
                            ps_y[(i, m0)][:],
                            lhsT=emb_sl(t, i * 128, (i + 1) * 128),
                            rhs=at[:, m0 : m0 + mw],
                            start=st,
                            stop=sp,
                        )

            deg_acc = cpool.tile([128, msh], mm_dt)
            b.vector.tensor_add(deg_acc[:], deg_p[0][:], deg_p[1][:])
            deg_acc2 = cpool.tile([128, msh], mm_dt)
            b.vector.tensor_add(deg_acc2[:], deg_p[2][:], deg_p[3][:])
            b.vector.tensor_add(deg_acc[:], deg_acc[:], deg_acc2[:])

            for (m0, mw) in mchunks:
                # x = embT + yT / deg, then row-normalize; per-node scalars are
                # broadcast across partitions with a K=1 ones matmul.
                ps_d = bpool.tile([1, mw], F32, tag="psd")
                b.tensor.matmul(
                    ps_d[:1, :],
                    lhsT=ones_col_mm[:, :1],
                    rhs=deg_acc[:, m0 : m0 + mw],
                    start=True,
                    stop=True,
                )
                rinv = wpool.tile([1, mw], F32, tag="rinv")
                b.vector.tensor_scalar_add(rinv[:1, :], ps_d[:1, :], 1e-6)
                b.vector.reciprocal(rinv[:1, :], rinv[:1, :])
                rinv_bp = bpool.tile([128, mw], F32, tag="bc")
                b.tensor.matmul(
                    rinv_bp[:], lhsT=ones_row[:1, :], rhs=rinv[:1, :], start=True, stop=True
                )
                rinv_b = wpool.tile([128, mw], F32, tag="rinvb")
                b.scalar.copy(rinv_b[:], rinv_bp[:])
                xts = []
                for i in range(ndt):
                    ebt = spool.tile([128, mw], F32, tag="ebt")
                    b.sync.dma_start(
                        out=ebt[:], in_=embT.ap()[128 * i : 128 * (i + 1), m0 : m0 + mw]
                    )
                    xt = wpool.tile([128, mw], F32, tag=f"xt{i}")
                    b.vector.tensor_mul(xt[:], ps_y[(i, m0)][:], rinv_b[:])
                    b.vector.tensor_add(xt[:], xt[:], ebt[:])
                    xts.append(xt)
                ns = bpool.tile([1, mw], F32, tag="bc")
                for i in range(ndt):
                    sq = wpool.tile([128, mw], F32, tag="sq")
                    b.scalar.square(sq[:], xts[i][:])
                    b.tensor.matmul(
                        ns[:1, :],
                        lhsT=ones_col[:, :1],
                        rhs=sq[:],
                        start=(i == 0),
                        stop=(i == ndt - 1),
                    )
                nrm = wpool.tile([1, mw], F32, tag="nrm")
                b.scalar.sqrt(nrm[:1, :], ns[:1, :])
                b.vector.tensor_scalar_max(nrm[:1, :], nrm[:1, :], 1e-8)
                rn = wpool.tile([1, mw], F32, tag="rn")
                b.vector.reciprocal(rn[:1, :], nrm[:1, :])
                rn_bp = bpool.tile([128, mw], F32, tag="bc")
                b.tensor.matmul(
                    rn_bp[:], lhsT=ones_row[:1, :], rhs=rn[:1, :], start=True, stop=True
                )
                rn_b = wpool.tile([128, mw], F32, tag="rnb")
                b.scalar.copy(rn_b[:], rn_bp[:])
                for i in range(ndt):
                    xn = wpool.tile([128, mw], out_dt, tag="xn")
                    b.vector.tensor_mul(xn[:], xts[i][:], rn_b[:])
                    b.sync.dma_start(
                        out=xnT.ap()[128 * i : 128 * (i + 1), m0 : m0 + mw], in_=xn[:]
                    )
    b.compile()
    return b


def build_stage2(
    n=N, d=D, q=Q, nc_cores=NC, pair=1024, dat_dt=F32, cce_mult=False, use_ttr=False
):
    # cce_mult: fold the src*dst mask product into the dst gather via the DMA
    # CCE ALU. Rejected by neuronx-cc ("DMACopy does not support mult with
    # Copy mode"), kept for reference; the DVE computes cn instead.
    # use_ttr: the fused InstTensorTensorReduce compiles but the NEFF fails at
    # runtime on HW (readback INTERNAL error); the unfused mul+reduce+add
    # path is the default.
    """Per-core: w [q/nc, 1] from gathered adj rows + resident xnT.

    Whole adjacency rows are gathered per edge-tile with one indirect DMA per
    matrix; the src*dst mask product is computed by the DMA's inline CCE
    multiply (exact for 0/1 data).  cos tiles are accumulated into 2-bank
    PSUM pairs and consumed by two wide DVE passes (mul + fused mul-reduce).
    """
    ql = q // nc_cores
    etw = min(128, ql)
    net = ql // etw
    r = 2 * ql
    ndt = d // 128

    b = bacc.Bacc(
        "TRN2",
        target_bir_lowering=False,
        debug=False,
        num_devices=nc_cores,
        dynamic_dma_scratch_size=65536,
    )
    xnTf = b.dram_tensor("xnTf", [d, n], dat_dt, kind="ExternalInput")
    tbl = b.dram_tensor("tbl", [r, n], dat_dt, kind="ExternalInput")
    idxs = b.dram_tensor("idxs", [ql, 1], I32, kind="ExternalInput")
    idxd = b.dram_tensor("idxd", [ql, 1], I32, kind="ExternalInput")
    ut = b.dram_tensor("ut", [d, ql], dat_dt, kind="ExternalInput")
    vt = b.dram_tensor("vt", [d, ql], dat_dt, kind="ExternalInput")
    w = b.dram_tensor("w", [ql, 1], F32, kind="ExternalOutput")

    MMW = 512  # matmul moving-dim / PSUM bank width (fp32 out)

    with tile.TileContext(b) as tc:
        with (
            tc.tile_pool(name="const", bufs=1) as cpool,
            tc.tile_pool(name="gather", bufs=2) as gpool,
            tc.tile_pool(name="mid", bufs=2) as mpool,
            tc.tile_pool(name="small", bufs=2) as wpool,
            tc.tile_pool(name="cos", bufs=2, space="PSUM") as ppool,
        ):
            ix_s, ix_d = [], []
            for et in range(net):
                ts_ = cpool.tile([etw, 1], I32, tag=f"ixs{et}")
                b.sync.dma_start(out=ts_[:], in_=idxs.ap()[et * etw : (et + 1) * etw, :1])
                ix_s.append(ts_)
                td_ = cpool.tile([etw, 1], I32, tag=f"ixd{et}")
                b.sync.dma_start(out=td_[:], in_=idxd.ap()[et * etw : (et + 1) * etw, :1])
                ix_d.append(td_)

            def gather_pair(et):
                aS = gpool.tile([etw, n], dat_dt, tag="aS", bufs=3, name=f"aS{et}")
                b.gpsimd.indirect_dma_start(
                    out=aS[:],
                    out_offset=None,
                    in_=tbl.ap(),
                    in_offset=bass.IndirectOffsetOnAxis(ap=ix_s[et][:, :1], axis=0),
                )
                aD = gpool.tile([etw, n], dat_dt, tag="aD", bufs=2, name=f"aD{et}")
                b.gpsimd.indirect_dma_start(
                    out=aD[:],
                    out_offset=None,
                    in_=tbl.ap(),
                    in_offset=bass.IndirectOffsetOnAxis(ap=ix_d[et][:, :1], axis=0),
                )
                return aS, aD

            pend = {0: gather_pair(0)}

            XCH = 2048  # pair (1024) always falls inside one chunk
            xchunks = _chunks(n, XCH)
            xn_sb = {}
            for i in range(ndt):
                for (c0, cwd) in xchunks:
                    t_ = cpool.tile(
                        [128, cwd], dat_dt, tag=f"xn{i}_{c0}", name=f"xn{i}_{c0}"
                    )
                    b.sync.dma_start(
                        out=t_[:], in_=xnTf.ap()[128 * i : 128 * (i + 1), c0 : c0 + cwd]
                    )
                    xn_sb[(i, c0)] = t_

            def xn_sl(i, lo, hi):
                c0 = (lo // XCH) * XCH
                t_ = xn_sb[(i, c0)]
                return t_[:, lo - c0 : hi - c0]
            ut_sb, vt_sb = [], []
            for i in range(ndt):
                tu = cpool.tile([128, ql], dat_dt, tag=f"ut{i}")
                b.sync.dma_start(out=tu[:], in_=ut.ap()[128 * i : 128 * (i + 1), :])
                ut_sb.append(tu)
                tv = cpool.tile([128, ql], dat_dt, tag=f"vt{i}")
                b.sync.dma_start(out=tv[:], in_=vt.ap()[128 * i : 128 * (i + 1), :])
                vt_sb.append(tv)


            for et in range(net):
                esl = slice(et * etw, (et + 1) * etw)
                aS, aD = pend.pop(et)
                half = n // 2
                b.gpsimd.tensor_mul(aS[:, :half], aS[:, :half], aD[:, :half])
                b.gpsimd.tensor_mul(aS[:, half:], aS[:, half:], aD[:, half:])
                cn = aS
                if et + 1 < net:
                    pend[et + 1] = gather_pair(et + 1)

                npair = len(_chunks(n, pair))
                parts = wpool.tile([etw, npair], F32, tag="parts")
                for pi, (c0, cwi) in enumerate(_chunks(n, pair)):
                    cosR = ppool.tile([etw, cwi], F32, tag="cosR")
                    cosL = ppool.tile([etw, cwi], F32, tag="cosL")
                    for i in range(ndt):
                        st, sp = (i == 0), (i == ndt - 1)
                        for (h0, hw) in _chunks(cwi, MMW):
                            b.tensor.matmul(
                                cosR[:, h0 : h0 + hw],
                                lhsT=vt_sb[i][:, esl],
                                rhs=xn_sl(i, c0 + h0, c0 + h0 + hw),
                                start=st,
                                stop=sp,
                            )
                            b.tensor.matmul(
                                cosL[:, h0 : h0 + hw],
                                lhsT=ut_sb[i][:, esl],
                                rhs=xn_sl(i, c0 + h0, c0 + h0 + hw),
                                start=st,
                                stop=sp,
                            )
                    m1 = mpool.tile([etw, cwi], F32, tag="m1")
                    b.vector.tensor_mul(m1[:], cn[:, c0 : c0 + cwi], cosR[:])
                    m2 = mpool.tile([etw, cwi], F32, tag="m2")
                    b.vector.tensor_mul(m2[:], m1[:], cosL[:])
                    # row-sum on the scalar engine (accum_out), freeing DVE;
                    # identity copy in place so no scratch tile is needed
                    b.scalar.activation(
                        m2[:],
                        m2[:],
                        AF.Copy,
                        accum_out=parts[:, pi : pi + 1],
                    )
                wacc = wpool.tile([etw, 1], F32, tag="wacc")
                b.vector.reduce_sum(wacc[:, :1], parts[:], axis=mybir.AxisListType.X)
                sg = wpool.tile([etw, 1], F32, tag="sg")
                b.scalar.activation(sg[:, :1], wacc[:, :1], AF.Sigmoid)
                b.sync.dma_start(out=w.ap()[et * etw : (et + 1) * etw, :1], in_=sg[:, :1])
    b.compile()
    return b


def make_stage1_inputs(emb, adj, n=N, d=D, nc_cores=NC, mm_np=np.float32):
    msh = n // nc_cores
    kt = (n + 127) // 128
    kp = kt * 128
    dst = d + 1
    e_pad = np.zeros((kp, dst), mm_np)
    e_pad[:n, :d] = emb.astype(mm_np)
    e_pad[:n, d] = 1.0
    embx = np.ascontiguousarray(
        e_pad.reshape(kt, 128, dst).transpose(1, 0, 2).reshape(128, kt * dst)
    )
    ins = []
    for k in range(nc_cores):
        sh = adj[k * msh : (k + 1) * msh, :]
        adjT = np.zeros((kp, msh), mm_np)
        adjT[:n] = sh.T.astype(mm_np)
        embT = np.ascontiguousarray(emb[k * msh : (k + 1) * msh, :].T)
        ins.append({"adjT": adjT, "embx": embx, "embT": embT})
    return ins


def make_stage2_inputs(adj, xnT, src, dst_, n=N, q=Q, nc_cores=NC, dat_np=np.float32):
    ql = q // nc_cores
    ins = []
    for k in range(nc_cores):
        s_k = src[k * ql : (k + 1) * ql]
        d_k = dst_[k * ql : (k + 1) * ql]
        uniq = np.unique(np.concatenate([s_k, d_k]))
        tbl = np.zeros((2 * ql, n), dat_np)
        tbl[: len(uniq)] = adj[uniq].astype(dat_np)
        ins.append(
            {
                "xnTf": xnT,
                "tbl": tbl,
                "idxs": np.searchsorted(uniq, s_k).astype(np.int32)[:, None],
                "idxd": np.searchsorted(uniq, d_k).astype(np.int32)[:, None],
                "ut": np.ascontiguousarray(xnT[:, s_k]),
                "vt": np.ascontiguousarray(xnT[:, d_k]),
            }
        )
    return ins


_progs = {}
LAST_RESULTS = []  # BassKernelResults of the most recent kernel() call (for profiling)


def _get(name, builder):
    if name not in _progs:
        _progs[name] = builder()
    return _progs[name]


def kernel(emb_weight, adj, edges):
    emb = np.asarray(emb_weight, dtype=np.float32)
    adj = np.asarray(adj, dtype=np.float32)
    edges = np.asarray(edges)
    src = edges[0].astype(np.int64)
    dst_ = edges[1].astype(np.int64)

    if USE_BF16:
        mm_dt, out_dt, dat_dt = BF16, BF16, BF16
        mm_np = dat_np = NP_BF16
    else:
        mm_dt, out_dt, dat_dt = F32, F32, F32
        mm_np = dat_np = np.float32
    s1 = _get("s1", lambda: build_stage1(mm_dt=mm_dt, out_dt=out_dt))
    s2 = _get("s2", lambda: build_stage2(dat_dt=dat_dt))

    in1 = make_stage1_inputs(emb, adj, mm_np=mm_np)
    r1 = bass_utils.run_bass_kernel_spmd(s1, in1, core_ids=list(range(NC)))
    xnT = np.concatenate([r1.results[k]["xnT"] for k in range(NC)], axis=1)

    in2 = make_stage2_inputs(adj, xnT, src, dst_, dat_np=dat_np)
    r2 = bass_utils.run_bass_kernel_spmd(s2, in2, core_ids=list(range(NC)))
    w = np.concatenate([r2.results[k]["w"][:, 0] for k in range(NC)])

    LAST_RESULTS.clear()
    LAST_RESULTS.extend([r1, r2])
    return w.astype(np.float32)



# revision 13
# speedup vs baseline: 1.0897x; 1.0897x over previous
"""CommonNeighborsPredictor kernel for 8 Trainium2 NeuronCores (fp8 rewrite).

Math (see reference):
    deg = adj.sum(-1) + 1e-6
    x   = emb + (adj @ emb) / deg[:, None]
    xn  = x / max(||x||_2, 1e-8)
    w_e = sum_c adj[src_e,c] * adj[dst_e,c] * (xn[src_e]@xn[c]) * (xn[dst_e]@xn[c])
    out = sigmoid(w)

Distribution: 2 SPMD launches, no collectives.
  Stage 1 shards nodes (rows of adj) 8 ways; core k computes xnT for its
    1250 nodes.  All matmuls are fp8e4 DoubleRow (K=256 per instruction):
    lhsT = emb d-chunk [128,2,128], moving = adjT k-tile [128,2,m].
    Degrees come from an extra M=1 ones-matmul per k-tile, col-tiled so the
    three m-chunk accumulators share one PSUM bank (only the first issues
    start=True, so the bank's has_written bits are cleared exactly once).
    The epilogue packs the per-node scalars into [128,n] via small
    SBUF->SBUF reshape DMAs so the iterative DVE reciprocal runs on all 128
    lanes, then broadcasts them across partitions with K=1 ones matmuls.
  Stage 2 shards query edges 8 ways (512 each).  Adjacency rows are gathered
    per edge-tile from a dedup'd fp8 row table; the dst gather uses the
    SWDGE CCE inline add (+ fp8->bf16 cast) so cn01 = aS+aD lands in SBUF
    with zero engine work; the 0/1 common-neighbor mask is relu(cn01-1)
    (one DVE tensor_scalar, 4x mode, in place).  cos tiles are fp8
    DoubleRow matmuls (single K=256 group -> one PSUM write per bank).  The
    weighted reduction is m1 = mask*cosR (DVE), then one fused
    scalar_tensor_tensor multiply with accum_out (DVE) - no separate reduce
    pass.  A tunable subset of chunks has cosR/cosL pre-evacuated to bf16
    SBUF by the scalar engine to balance DVE vs ACT load.

dtypes: adjacency and xn are fp8e4 (adjacency 0/1/2 values are exact; xn
rounding contributes ~1e-3 abs error on the output vs the 2e-2 gate).
PSUM accumulation and all per-node scalar math are fp32.
"""

import numpy as np

import concourse.bass as bass
import concourse.bacc as bacc
import concourse.mybir as mybir
import concourse.tile as tile
from concourse import bass_utils

F32 = mybir.dt.float32
BF16 = mybir.dt.bfloat16
FP8 = mybir.dt.float8e4
I32 = mybir.dt.int32
AF = mybir.ActivationFunctionType
OP = mybir.AluOpType
DR = mybir.MatmulPerfMode.DoubleRow
NP_FP8 = mybir.dt.np(FP8)
NP_BF16 = mybir.dt.np(BF16)

N, D, Q, NC = 10000, 256, 4096, 8
MSH = N // NC            # 1250 nodes per core (stage 1)
NKT = 40                 # k-tiles of 256 (contraction padded to 10240)
KP = NKT * 256
QL = Q // NC             # 512 edges per core (stage 2)
ETW = 128                # edge-tile width
NET = QL // ETW          # 4 edge tiles
N2 = 10240               # padded node count for stage 2
CHW = 1024               # stage-2 chunk width (psum cos tile)
NCH = N2 // CHW          # 10 chunks per edge tile

MSHP = 1280              # adjT row stride (16-aligned for DoubleRow APs)
MCH = [(0, 512), (512, 512), (1024, 226)]  # stage-1 m-chunks

# stage-2 tuning: chunks with (et*NCH+ci) % EVAC2_MOD == 0 get both cos
# tiles evacuated to bf16 by ACT (type-2); all others get only cosL
# evacuated (type-1).  0 disables type-2.
EVAC2_MOD = 4
MASK_ON_ACT = False


def build_stage1():
    b = bacc.Bacc("TRN2", target_bir_lowering=False, debug=False, num_devices=NC)
    adjx = b.dram_tensor("adjx", [128, NKT * 2 * MSHP], FP8, kind="ExternalInput")
    embx = b.dram_tensor("embx", [128, NKT * 2 * D], FP8, kind="ExternalInput")
    embT = b.dram_tensor("embT", [128, 2 * MSH], F32, kind="ExternalInput")
    xnT = b.dram_tensor("xnT", [D, MSH], FP8, kind="ExternalOutput")

    AKT = 4   # k-tiles per adjx DMA chunk (1.28 MB)
    nach = NKT // AKT

    with tile.TileContext(b) as tc:
        with (
            tc.tile_pool(name="const", bufs=1) as cpool,
            tc.tile_pool(name="adj", bufs=3) as apool,
            tc.tile_pool(name="work", bufs=1) as wpool,
            tc.tile_pool(name="acc", bufs=1, space="PSUM") as ppool,
        ):
            emb_sb = cpool.tile([128, NKT * 2, D], FP8)
            # split the emb upload so the first k-tiles' weights arrive fast
            b.sync.dma_start(out=emb_sb[:, : 2 * AKT, :],
                             in_=embx.ap()[:, : 2 * AKT * D])
            adj_t = [None] * nach

            def adj_dma(ci):
                t_ = apool.tile([128, 2 * AKT, MSHP], FP8, tag="adjx", name=f"adj{ci}")
                b.sync.dma_start(
                    out=t_[:],
                    in_=adjx.ap()[:, ci * 2 * AKT * MSHP : (ci + 1) * 2 * AKT * MSHP],
                )
                adj_t[ci] = t_

            adj_dma(0)
            adj_dma(1)
            b.sync.dma_start(out=emb_sb[:, 2 * AKT :, :],
                             in_=embx.ap()[:, 2 * AKT * D :])
            ones2 = cpool.tile([128, 2, 16], FP8)
            b.vector.memset(ones2[:], 1.0)
            ones_row = cpool.tile([1, 128], F32)
            b.vector.memset(ones_row[:1, :], 1.0)
            ones_col_bf = cpool.tile([128, 1], BF16)
            b.vector.memset(ones_col_bf[:, :1], 1.0)
            embT_sb = cpool.tile([128, 2 * MSH], F32)
            b.sync.dma_start(out=embT_sb[:], in_=embT.ap())

            ps_y = {}
            for i in range(2):
                for (m0, mw) in MCH[:2]:
                    ps_y[(i, m0)] = ppool.tile(
                        [128, mw], F32, tag=f"py{i}_{m0}", name=f"py{i}_{m0}")
            # the two narrow m-chunk accumulators share one bank (2x226 fp32)
            ps_y3 = ppool.tile([128, 452], F32, tag="py_1024")
            ps_y[(0, 1024)] = ps_y3[:, 0:226]
            ps_y[(1, 1024)] = ps_y3[:, 226:452]
            ps_deg = [
                ppool.tile([128, 512], F32, tag=f"pdeg{mc}", name=f"pdeg{mc}")
                for mc in range(3)
            ]

            for t in range(NKT):
                ci = t // AKT
                if ci + 1 < nach and adj_t[ci + 1] is None:
                    adj_dma(ci + 1)
                at = adj_t[ci]
                j = 2 * (t % AKT)
                st, sp = (t == 0), (t == NKT - 1)
                for i in range(2):
                    lhs = emb_sb[:, 2 * t : 2 * t + 2, 128 * i : 128 * (i + 1)]
                    for mc, (m0, mw) in enumerate(MCH):
                        b.tensor.matmul(
                            ps_y[(i, m0)][:, :mw],
                            lhsT=lhs,
                            rhs=at[:, j : j + 2, m0 : m0 + mw],
                            # the shared bank is cleared once, by (0,1024)
                            start=(st and not (i == 1 and mc == 2)),
                            stop=sp,
                            perf_mode=DR,
                            skip_group_check=(i == 1 and mc == 2),
                        )
                for mc, (m0, mw) in enumerate(MCH):
                    b.tensor.matmul(
                        ps_deg[mc][0:1, :mw],
                        lhsT=ones2[:, :, :1],
                        rhs=at[:, j : j + 2, m0 : m0 + mw],
                        start=st,
                        stop=sp,
                        perf_mode=DR,
                    )

            # ---- epilogue ----
            # pack deg [1,mw]x3 (partitions 0/32/64) -> [128,12] so the
            # iterative reciprocal runs on all 128 lanes.
            degs = wpool.tile([1, 1536], F32, tag="degs")
            b.vector.memset(degs[:1, :], 1.0)
            for mc, (m0, mw) in enumerate(MCH):
                b.scalar.copy(degs[0:1, 512 * mc : 512 * mc + mw],
                              ps_deg[mc][0:1, :mw])
            rinvp = wpool.tile([128, 12], F32, tag="rinvp")
            for mc in range(3):
                b.sync.dma_start(out=rinvp[:, 4 * mc : 4 * mc + 4],
                                 in_=degs[0:1, 512 * mc : 512 * mc + 512])
            b.vector.tensor_scalar_add(rinvp[:], rinvp[:], 1e-6)
            b.vector.reciprocal(rinvp[:], rinvp[:])
            rinv_row = wpool.tile([1, 1536], F32, tag="rinvr")
            for mc in range(3):
                b.sync.dma_start(out=rinv_row[:1, 512 * mc : 512 * mc + 512],
                                 in_=rinvp[:, 4 * mc : 4 * mc + 4])

            xts = {}
            for mc, (m0, mw) in enumerate(MCH):
                rb_ps = ppool.tile([128, 512], F32, tag=f"pdeg{mc}", name="rb_ps")
                b.tensor.matmul(
                    rb_ps[:, :mw], lhsT=ones_row[:1, :],
                    rhs=rinv_row[:1, m0 : m0 + mw], start=True, stop=True,
                )
                rb = wpool.tile([128, mw], F32, tag=f"rb{mc}")
                b.scalar.copy(rb[:], rb_ps[:, :mw])
                for i in range(2):
                    xt = wpool.tile([128, mw], F32, tag=f"xt{i}_{mc}")
                    b.vector.tensor_mul(xt[:], ps_y[(i, m0)][:, :mw], rb[:])
                    b.vector.tensor_add(
                        xt[:], xt[:], embT_sb[:, i * MSH + m0 : i * MSH + m0 + mw])
                    xts[(i, mc)] = xt
            ns_ps = {}
            for mc, (m0, mw) in enumerate(MCH):
                ns_ps[mc] = ppool.tile([128, 512], F32, tag=f"pdeg{mc}", name="ns_ps")
                for i in range(2):
                    sq = wpool.tile([128, mw], BF16, tag=f"sq{i}_{mc}")
                    b.scalar.square(sq[:], xts[(i, mc)][:])
                    b.tensor.matmul(
                        ns_ps[mc][0:1, :mw],
                        lhsT=ones_col_bf[:, :1],
                        rhs=sq[:],
                        start=(i == 0),
                        stop=(i == 1),
                    )
            nss = wpool.tile([1, 1536], F32, tag="nss")
            b.vector.memset(nss[:1, :], 1.0)
            for mc, (m0, mw) in enumerate(MCH):
                b.scalar.copy(nss[0:1, 512 * mc : 512 * mc + mw],
                              ns_ps[mc][0:1, :mw])
            rnp = wpool.tile([128, 12], F32, tag="rnp")
            for mc in range(3):
                b.sync.dma_start(out=rnp[:, 4 * mc : 4 * mc + 4],
                                 in_=nss[0:1, 512 * mc : 512 * mc + 512])
            b.scalar.sqrt(rnp[:], rnp[:])
            b.vector.tensor_scalar_max(rnp[:], rnp[:], 1e-8)
            b.vector.reciprocal(rnp[:], rnp[:])
            rn_row = wpool.tile([1, 1536], F32, tag="rnr")
            for mc in range(3):
                b.sync.dma_start(out=rn_row[:1, 512 * mc : 512 * mc + 512],
                                 in_=rnp[:, 4 * mc : 4 * mc + 4])
            for mc, (m0, mw) in enumerate(MCH):
                rnb_ps = ppool.tile([128, 512], F32, tag=f"pdeg{mc}", name="rnb_ps")
                b.tensor.matmul(
                    rnb_ps[:, :mw], lhsT=ones_row[:1, :],
                    rhs=rn_row[:1, m0 : m0 + mw], start=True, stop=True,
                )
                rnb = wpool.tile([128, mw], F32, tag=f"rnb{mc}")
                b.scalar.copy(rnb[:], rnb_ps[:, :mw])
                for i in range(2):
                    xn = wpool.tile([128, mw], FP8, tag=f"xn{i}_{mc}")
                    b.vector.tensor_mul(xn[:], xts[(i, mc)][:], rnb[:])
                    b.sync.dma_start(
                        out=xnT.ap()[128 * i : 128 * (i + 1), m0 : m0 + mw],
                        in_=xn[:],
                    )
    b.compile()
    return b


def build_stage2():
    b = bacc.Bacc(
        "TRN2", target_bir_lowering=False, debug=False, num_devices=NC,
        dynamic_dma_scratch_size=65536,
    )
    xnx = b.dram_tensor("xnx", [128, 2 * N2], FP8, kind="ExternalInput")
    tbl = b.dram_tensor("tbl", [2 * QL, N2], FP8, kind="ExternalInput")
    idxs = b.dram_tensor("idxs", [QL, 1], I32, kind="ExternalInput")
    idxd = b.dram_tensor("idxd", [QL, 1], I32, kind="ExternalInput")
    utx = b.dram_tensor("utx", [128, 2 * QL], FP8, kind="ExternalInput")
    vtx = b.dram_tensor("vtx", [128, 2 * QL], FP8, kind="ExternalInput")
    w = b.dram_tensor("w", [QL, 1], F32, kind="ExternalOutput")

    with tile.TileContext(b) as tc:
        with (
            tc.tile_pool(name="const", bufs=1) as cpool,
            tc.tile_pool(name="gather", bufs=2) as gpool,
            tc.tile_pool(name="mid", bufs=2) as mpool,
            tc.tile_pool(name="small", bufs=2) as wpool,
            tc.tile_pool(name="cos", bufs=2, space="PSUM") as ppool,
        ):
            ix_s, ix_d = [], []
            for et in range(NET):
                ts_ = cpool.tile([ETW, 1], I32, tag=f"ixs{et}")
                b.sync.dma_start(out=ts_[:], in_=idxs.ap()[et * ETW : (et + 1) * ETW, :1])
                ix_s.append(ts_)
                td_ = cpool.tile([ETW, 1], I32, tag=f"ixd{et}")
                b.sync.dma_start(out=td_[:], in_=idxd.ap()[et * ETW : (et + 1) * ETW, :1])
                ix_d.append(td_)

            def gather_cn(et):
                # two plain fp8->bf16 cast gathers; the CCE-add variant races
                # with the base gather (silent wrong masks) - combined on DVE.
                gs = gpool.tile([ETW, N2], BF16, tag="cns", bufs=2, name=f"cns{et}")
                b.gpsimd.indirect_dma_start(
                    out=gs[:], out_offset=None, in_=tbl.ap(),
                    in_offset=bass.IndirectOffsetOnAxis(ap=ix_s[et][:, :1], axis=0),
                )
                gd = gpool.tile([ETW, N2], BF16, tag="cnd", bufs=2, name=f"cnd{et}")
                b.gpsimd.indirect_dma_start(
                    out=gd[:], out_offset=None, in_=tbl.ap(),
                    in_offset=bass.IndirectOffsetOnAxis(ap=ix_d[et][:, :1], axis=0),
                )
                return gs, gd

            pend = {0: gather_cn(0)}

            xn_sb = cpool.tile([128, 2, N2], FP8)
            HCH = N2 // 4
            for ko in range(2):
                for h in range(4):
                    b.sync.dma_start(
                        out=xn_sb[:, ko : ko + 1, h * HCH : (h + 1) * HCH],
                        in_=xnx.ap()[:, ko * N2 + h * HCH : ko * N2 + (h + 1) * HCH],
                    )
            ut_sb = cpool.tile([128, 2, QL], FP8)
            b.sync.dma_start(out=ut_sb[:], in_=utx.ap())
            vt_sb = cpool.tile([128, 2, QL], FP8)
            b.sync.dma_start(out=vt_sb[:], in_=vtx.ap())

            for et in range(NET):
                esl = slice(et * ETW, (et + 1) * ETW)
                gs, gd = pend.pop(et)
                # cn = aS + aD - 1; the relu folds into the m1 pass below
                cn = gpool.tile([ETW, N2], BF16, tag="cn", bufs=2, name=f"cn{et}")
                b.vector.scalar_tensor_tensor(
                    out=cn[:], in0=gs[:], scalar=-1.0, in1=gd[:],
                    op0=OP.add, op1=OP.add)
                if et + 1 < NET:
                    pend[et + 1] = gather_cn(et + 1)

                parts = wpool.tile([ETW, NCH], F32, tag="parts")
                for ci in range(NCH):
                    c0 = ci * CHW
                    cosR = ppool.tile([ETW, CHW], F32, tag="cosR")
                    cosL = ppool.tile([ETW, CHW], F32, tag="cosL")
                    for h in range(2):
                        hsl = slice(c0 + h * 512, c0 + (h + 1) * 512)
                        b.tensor.matmul(
                            cosR[:, h * 512 : (h + 1) * 512],
                            lhsT=vt_sb[:, :, esl], rhs=xn_sb[:, :, hsl],
                            start=True, stop=True, perf_mode=DR,
                        )
                        b.tensor.matmul(
                            cosL[:, h * 512 : (h + 1) * 512],
                            lhsT=ut_sb[:, :, esl], rhs=xn_sb[:, :, hsl],
                            start=True, stop=True, perf_mode=DR,
                        )
                    # fp32 chain: bf16 product rounding costs ~1e-2 on the
                    # max-err gate; fp32 keeps the pipeline at the fp8-input
                    # error floor (~3e-4).
                    m1 = mpool.tile([ETW, CHW], F32, tag="m1")
                    b.vector.scalar_tensor_tensor(
                        out=m1[:], in0=cn[:, c0 : c0 + CHW], scalar=0.0,
                        in1=cosR[:], op0=OP.max, op1=OP.mult)
                    junk = mpool.tile([ETW, CHW], F32, tag="junk")
                    b.vector.scalar_tensor_tensor(
                        out=junk[:], in0=m1[:], scalar=1.0, in1=cosL[:],
                        op0=OP.mult, op1=OP.mult,
                        accum_out=parts[:, ci : ci + 1],
                    )
                wacc = wpool.tile([ETW, 1], F32, tag="wacc")
                b.vector.reduce_sum(wacc[:, :1], parts[:], axis=mybir.AxisListType.X)
                sg = wpool.tile([ETW, 1], F32, tag="sg")
                b.scalar.activation(sg[:, :1], wacc[:, :1], AF.Sigmoid)
                b.sync.dma_start(out=w.ap()[et * ETW : (et + 1) * ETW, :1], in_=sg[:, :1])
    b.compile()
    return b


# ---------------- host-side data prep ----------------

def dr_pack(a, part=128):
    """[K, M] -> [part, K//(2*part) * 2 * M] DoubleRow layout."""
    k, m = a.shape
    nkt = k // (2 * part)
    return np.ascontiguousarray(
        a.reshape(nkt, 2, part, m).transpose(2, 0, 1, 3).reshape(part, nkt * 2 * m)
    )


def make_stage1_inputs(emb, adj):
    emb8 = emb.astype(NP_FP8)
    e_pad = np.zeros((KP, D), NP_FP8)
    e_pad[:N] = emb8
    embx = dr_pack(e_pad)
    ins = []
    for k in range(NC):
        sh8 = adj[k * MSH : (k + 1) * MSH, :].astype(NP_FP8)  # [MSH, N] exact
        adjT = np.zeros((KP, MSHP), NP_FP8)
        adjT[:N, :MSH] = sh8.T
        adjx = dr_pack(adjT)
        embT = np.ascontiguousarray(
            emb[k * MSH : (k + 1) * MSH, :].T.reshape(2, 128, MSH)
            .transpose(1, 0, 2).reshape(128, 2 * MSH).astype(np.float32)
        )
        ins.append({"adjx": adjx, "embx": embx, "embT": embT})
    return ins


def make_stage2_inputs(adj8, xnT, src, dst_):
    """adj8: [N, N] fp8; xnT: [D, N] fp8 (from stage 1)."""
    xp = np.zeros((D, N2), NP_FP8)
    xp[:, :N] = xnT
    xnx = np.ascontiguousarray(
        xp.reshape(2, 128, N2).transpose(1, 0, 2).reshape(128, 2 * N2))
    ins = []
    for k in range(NC):
        s_k = src[k * QL : (k + 1) * QL]
        d_k = dst_[k * QL : (k + 1) * QL]
        uniq = np.unique(np.concatenate([s_k, d_k]))
        tbl = np.zeros((2 * QL, N2), NP_FP8)
        tbl[: len(uniq), :N] = adj8[uniq]
        ut = np.ascontiguousarray(
            xp[:, s_k].reshape(2, 128, QL).transpose(1, 0, 2).reshape(128, 2 * QL))
        vt = np.ascontiguousarray(
            xp[:, d_k].reshape(2, 128, QL).transpose(1, 0, 2).reshape(128, 2 * QL))
        ins.append({
            "xnx": xnx,
            "tbl": tbl,
            "idxs": np.searchsorted(uniq, s_k).astype(np.int32)[:, None],
            "idxd": np.searchsorted(uniq, d_k).astype(np.int32)[:, None],
            "utx": ut,
            "vtx": vt,
        })
    return ins


_progs = {}
LAST_RESULTS = []  # BassKernelResults of the most recent kernel() call


def _get(name, builder):
    if name not in _progs:
        _progs[name] = builder()
    return _progs[name]


def kernel(emb_weight, adj, edges):
    emb = np.asarray(emb_weight, dtype=np.float32)
    adj = np.asarray(adj, dtype=np.float32)
    edges = np.asarray(edges)
    src = edges[0].astype(np.int64)
    dst_ = edges[1].astype(np.int64)

    s1 = _get("s1", build_stage1)
    s2 = _get("s2", build_stage2)

    in1 = make_stage1_inputs(emb, adj)
    r1 = bass_utils.run_bass_kernel_spmd(s1, in1, core_ids=list(range(NC)))
    xnT = np.concatenate([r1.results[k]["xnT"] for k in range(NC)], axis=1)

    adj8 = adj.astype(NP_FP8)
    in2 = make_stage2_inputs(adj8, xnT, src, dst_)
    r2 = bass_utils.run_bass_kernel_spmd(s2, in2, core_ids=list(range(NC)))
    w = np.concatenate([r2.results[k]["w"][:, 0] for k in range(NC)])

    LAST_RESULTS.clear()
    LAST_RESULTS.extend([r1, r2])
    return w.astype(np.float32)
